# revision 31
# baseline (speedup 1.0000x reference)
"""Trainium2 Bass kernel for nn_NeuralMemory (B=4, N=1024, D=128, DEPTH=4).

Sharding: 8 cores, core c handles batch b = c//2. Both cores of a pair
compute the store phase (per-token grads summed over all 1024 tokens)
redundantly -- the grad sum is order-invariant, so each core gets its
batch's sequence with its own retrieval half rotated to the front and
retrieves tokens [0:512) of its view. No collectives (a pair AllReduce
has a ~10us floor, worse than the duplicated compute).

v5 design notes (on top of v4):
  - Input DMA head: w0eff is prepended to S^T host-side and shipped as
    DMA1 = [w0eff | S^T tokens 0:512) (160KB) so layer 0 + the whole
    ACT chain starts at the first notify (~3.35us); DMA2 = the t1 half.
  - x0 is gone: host composes w0q = wq @ w0 so retrieval layer-1 term 1
    is a single matmul straight off S^T (saves a mm + a DVE evict).
  - Engine rebalance for the backward-mul drain (the old g muls were 6
    serialized 658ns DVE ops, landing g0 ~2us after the last dsilu):
    t1-half muls + both pg1 evicts run on the otherwise-idle Pool
    engine; DVE keeps the M-critical t0 chain, pg0 evicts, m_r, the
    u adds and the output evicts. ACT does only silu/dsilu/reload +
    retrieval silus.
  - All v4 disciplines kept: bf16 everywhere, per-half tiles for
    tile-granular deps, PSUM bank reuse plan, XBAR for a1..a3/g3/g2
    token-major copies, PE-transpose + evict for g0/g1, M/dW1 in their
    own PSUM groups, ACT-table order silu -> dsilu -> silu.
"""

import numpy as np
import ml_dtypes

import concourse.bass as bass
import concourse.mybir as mybir
import concourse.tile as tile
from concourse import bacc
from concourse.bass import ts
from concourse.bass_utils import run_bass_kernel_spmd

B, N, D = 4, 1024, 128
NCORES = 8
NT = 512            # tokens retrieved per core (half a batch)
TT = 512            # store-phase token tile
NTI = N // TT
NCHUNK = N // 128
RH = 256            # retrieval sub-tile

# bf16 weight packs:
#  wpbu (urgent): w1 | w2 | w3s=(2/D)w3 | wv_r=-(2/D)Wv | w0q=wq@w0
#                 | wkq_t=wq@Wk^T
#  wpbr (rest):   w1^T | w2^T | w3^T | ident
#  stp:           [w0eff=Wk@w0 | S^T]  (split into two DMAs, t0 first)

f32 = mybir.dt.float32
bf16 = mybir.dt.bfloat16

AF = mybir.ActivationFunctionType
ALU = mybir.AluOpType


def _build_program(reps=1):
    nc = bacc.Bacc(
        "TRN2",
        target_bir_lowering=False,
        debug=False,
        enable_asserts=False,
        num_devices=NCORES,
    )

    stp_dr = nc.dram_tensor("stp", [128, D + N], bf16, kind="ExternalInput").ap()
    stm_dr = nc.dram_tensor("s_tmb", [128, N], bf16, kind="ExternalInput").ap()
    wbu_dr = nc.dram_tensor("wpbu", [D, 6 * D], bf16, kind="ExternalInput").ap()
    wbr_dr = nc.dram_tensor("wpbr", [D, 4 * D], bf16, kind="ExternalInput").ap()
    wf_dr = nc.dram_tensor("wpf", [D, 3 * D], f32, kind="ExternalInput").ap()
    out_dr = nc.dram_tensor("out", [128, NT // 128, D], bf16, kind="ExternalOutput").ap()

    with tile.TileContext(nc) as tc:
        for _ in range(reps):
            _emit(tc, stp_dr, stm_dr, wbu_dr, wbr_dr, wf_dr, out_dr)

    nc.compile()
    return nc


def _emit(tc, stp_dr, stm_dr, wbu_dr, wbr_dr, wf_dr, out_dr):
    nc = tc.nc
    from contextlib import ExitStack

    from concourse.tile_rust import add_dep_helper as _dep  # type: ignore

    with ExitStack() as ctx:
        consts = ctx.enter_context(tc.tile_pool(name="consts", bufs=1))
        big = ctx.enter_context(tc.tile_pool(name="big", bufs=1))
        # PSUM: ha0,ha1,hb0,hb1,hc0,hc1 (1 bank each) + stg(2) = 8 banks
        pp = ctx.enter_context(tc.tile_pool(name="pp", bufs=1, space="PSUM"))

        def pbank(tag, name, shape=None, dt=f32):
            return pp.tile(shape or [128, TT], dt, tag=tag, bufs=1, name=name)

        def pstage(name, w=512, dt=f32):
            return pp.tile([128, w], dt, tag="stg", bufs=2, name=name)

        # ---- DMAs, ordered by need (all HWDGE on the sync queue);
        # DMA1 = [w0eff | S^T t0-half] gates the whole forward chain ----
        sta = big.tile([128, D + TT], bf16, tag="sta")   # w0eff | S^T[:, :512]
        stb = big.tile([128, TT], bf16, tag="stb")       # S^T[:, 512:]
        wpbu = consts.tile([D, 6 * D], bf16, tag="wpbu")
        wpbr = consts.tile([D, 4 * D], bf16, tag="wpbr")
        s_tmb = big.tile([128, NCHUNK, 128], bf16, tag="s_tmb")  # token-major
        wpf = consts.tile([D, 3 * D], f32, tag="wpf")
        nc.sync.dma_start(sta[:], stp_dr[:, 0 : D + TT])
        nc.sync.dma_start(stb[:], stp_dr[:, D + TT :])
        nc.sync.dma_start(wpbu[:], wbu_dr)
        nc.sync.dma_start(wpbr[:], wbr_dr)
        nc.sync.dma_start(
            s_tmb[:], stm_dr.rearrange("p (c d) -> p c d", d=128)
        )
        nc.sync.dma_start(wpf[:], wf_dr)

        w0eff = sta[:, 0:D]
        stt = [sta[:, D : D + TT], stb[:]]
        w1b = wpbu[:, ts(0, D)]
        w2b = wpbu[:, ts(1, D)]
        w3s = wpbu[:, ts(2, D)]
        wv_r = wpbu[:, ts(3, D)]
        w0q = wpbu[:, ts(4, D)]                        # wq @ w0
        wkq_t = wpbu[:, ts(5, D)]                      # wq @ Wk^T
        wt = [wpbr[:, ts(i, D)] for i in range(3)]     # w1^T,w2^T,w3^T
        ident_b = wpbr[:, ts(3, D)]
        w_f = [wpf[:, ts(i, D)] for i in range(3)]     # w1,w2,w3 f32

        # tiny scratch silu pulls the first ACT table load off the
        # critical path (runs during the DMAs)
        scr = consts.tile([128, 1], f32, tag="scr")
        scr2 = consts.tile([128, 1], f32, tag="scr2")
        nc.gpsimd.memset(scr[:], 0.0)
        nc.scalar.activation(scr2[:], scr[:], AF.Silu)

        # PE warm-up: start the HAM clock window early so real matmuls
        # hit full clock by ~3us
        wupa = consts.tile([128, 128], f32, tag="wupa")
        nc.gpsimd.memset(wupa[:], 0.0)
        wupp = pstage("wupp")
        for _ in range(4):
            nc.tensor.matmul(
                wupp[:, 0:128], wupa[:], wupa[:],
                skip_group_check=True,
            )

        # per-half persistent SBUF tensors (feature-major, bf16)
        def halves(pfx):
            return [
                big.tile([128, TT], bf16, name=f"{pfx}{t}", tag=f"{pfx}{t}")
                for t in range(NTI)
            ]

        a1 = halves("a1")
        a2 = halves("a2")
        a3 = halves("a3")
        sp0 = halves("sp0")
        sp1 = halves("sp1")
        sp2 = halves("sp2")
        g0 = halves("g0")
        g1 = halves("g1")
        g2 = halves("g2")
        g3 = halves("g3")   # (2/D)(H3 - V)

        silu_insts = []
        dsilu_insts = []

        # ---- forward: six single-bank H tiles; silu -> bf16 halves ----
        hb = {}
        for li, tag in ((0, "ha"), (1, "hb"), (2, "hc")):
            for t in range(NTI):
                hb[li, t] = pbank(f"{tag}{t}", f"h{li}_{t}")
        pt = big.tile([128, NT], bf16, tag="pt")
        l2mm = {}
        for t in range(NTI):
            nc.tensor.matmul(hb[0, t][:], w0eff, stt[t])
            silu_insts.append(nc.scalar.activation(a1[t][:], hb[0, t][:], AF.Silu))
            l2mm[t] = nc.tensor.matmul(hb[1, t][:], w1b, a1[t][:])
            silu_insts.append(nc.scalar.activation(a2[t][:], hb[1, t][:], AF.Silu))
            nc.tensor.matmul(hb[2, t][:], w2b, a2[t][:])
            silu_insts.append(nc.scalar.activation(a3[t][:], hb[2, t][:], AF.Silu))
            # H3 - V accumulated in one stage bank (wv_r is negated+scaled)
            h3 = pstage(f"h3_{t}")
            nc.tensor.matmul(h3[:], wv_r, stt[t], start=True, stop=False)
            nc.tensor.matmul(h3[:], w3s, a3[t][:], start=False, stop=True)
            nc.vector.tensor_copy(g3[t][:], h3[:])
            if t == 0:
                # pt = (wq Wk^T)^T S^T: independent matmul off S^T t0;
                # pinned after L1-t0 so it never sits in the forward's
                # critical PE slot (it's only needed by the backward-era
                # X1 stop matmuls)
                px = pstage("p_pt")
                ptmm = nc.tensor.matmul(px[:], wkq_t, stt[0])
                _dep(ptmm.ins, l2mm[0].ins, sync=False,
                     reason="PE order: forward L1-t0 before pt")
                nc.vector.tensor_copy(pt[:], px[:])

        # ---- dsilu after all silus (one table switch); t0 first so the
        # backward c-mm/mul chain (same order) never queues behind the
        # other half's dsilu ----
        for spd, li in ((sp2, 2), (sp1, 1), (sp0, 0)):
            for t in (0, 1):
                di = nc.scalar.activation(
                    spd[t][:], hb[li, t][:], AF.Derivative_silu
                )
                dsilu_insts.append(di)
        for di in dsilu_insts:
            _dep(di.ins, silu_insts[-1].ins, sync=False, reason="act-table order")

        # ---- backward: tile-1 first (it gates M); c tiles reuse the H
        # banks (freed in dsilu order, which matches mul consumption).
        # Engine split: the t1-half muls go to the idle Pool engine so
        # the M-critical t0 chain never queues on DVE. ----
        # c-bank plan: c2 -> the stage banks (free right after the g3
        # evicts, so the first backward mm never waits a dsilu WAR),
        # c1 -> h2's banks (free after the sp2 dsilus), c0 -> h1's banks.
        cb = {}
        for t in (0, 1):
            cb[2, t] = pstage(f"c2_{t}")
        for li, tag in ((1, "hc"), (0, "hb")):
            for t in (0, 1):
                cb[li, t] = pbank(f"{tag}{t}", f"c{li}_{t}")
        last_cmm = None
        for li, gin, gout, spd in (
            (2, g3, g2, sp2), (1, g2, g1, sp1), (0, g1, g0, sp0)
        ):
            for t in (0, 1):
                last_cmm = nc.tensor.matmul(cb[li, t][:], wt[li], gin[t][:])
                nc.vector.tensor_mul(gout[t][:], cb[li, t][:], spd[t][:])

        # reload the silu table during the dW phase, off the tail
        scr3 = consts.tile([128, 1], f32, tag="scr3")
        dummy = nc.scalar.activation(scr3[:], scr[:], AF.Silu)
        _dep(dummy.ins, dsilu_insts[-1].ins, sync=False, reason="act-table order")

        # ---- token-major copies: XBAR for a1..a3,g3,g2; PE path for g0/g1 --
        a_tm = [None] + [
            big.tile([128, NCHUNK, 128], bf16, name=f"atm{i}", tag=f"atm{i}")
            for i in (1, 2, 3)
        ]
        g_tm = [
            big.tile([128, NCHUNK, 128], bf16, name=f"gtm{i}", tag=f"gtm{i}")
            for i in range(4)
        ]
        for src, dst, horder in (
            (a1, a_tm[1], (0, 1)), (a2, a_tm[2], (0, 1)), (a3, a_tm[3], (0, 1)),
            (g3, g_tm[3], (0, 1)), (g2, g_tm[2], (0, 1)),
        ):
            for h in horder:
                nc.sync.dma_start_transpose(dst[:, 4 * h : 4 * h + 4], src[h][:])

        # g1 feeds dW1 -> u1 -> retrieval layer 2; PE transposes into the
        # freed hb banks, both halves evicted on Pool (order-pinned after
        # the Pool g0t1 mul so M is never delayed).
        pg1 = [None, None]
        for h in (0, 1):
            pg1[h] = pbank(f"hc{h}", f"pg1_{h}", shape=[128, TT], dt=bf16)
            for j in range(4):
                nc.tensor.matmul(
                    pg1[h][:, ts(j, 128)], g1[h][:, ts(j, 128)], ident_b,
                    is_transpose=True,
                )

        # g0: PE transposes + DVE half evicts (lowest latency on the
        # tail); separate stage tiles per half so the t1 transposes don't
        # wait on the t0 eviction (tile-granular WAR). t0's mul lands
        # first, so t0 transposes/evicts/M-chunks all go first.
        pg0 = [pstage(f"p_g0{h}", w=512, dt=bf16) for h in range(NTI)]
        last_pg0 = None
        for h in (0, 1):
            for j in range(4):
                last_pg0 = nc.tensor.matmul(
                    pg0[h][:, ts(j, 128)], g0[h][:, ts(j, 128)], ident_b,
                    is_transpose=True,
                )
        ev_g0h0 = nc.vector.tensor_copy(
            g_tm[0][:, 0:4], pg0[0][:].rearrange("p (c d) -> p c d", d=128)
        )
        # t1's evict is split in two so the M t1-chunk matmuls (the tail
        # gate) start after only half the copy has landed
        ev_g0h1a = nc.vector.tensor_copy(
            g_tm[0][:, 4:6], pg0[1][:, 0:256].rearrange("p (c d) -> p c d", d=128)
        )
        _dep(ev_g0h1a.ins, ev_g0h0.ins, sync=False, reason="DVE order: t0 evict first")
        ev_g0h1b = nc.vector.tensor_copy(
            g_tm[0][:, 6:8], pg0[1][:, 256:512].rearrange("p (c d) -> p c d", d=128)
        )
        _dep(ev_g0h1b.ins, ev_g0h1a.ins, sync=False, reason="DVE order")

        # pg1 evicts on ACT right after the dsilus and BEFORE the silu
        # reload (Copy is in every table, so no extra load) -- dW1's
        # inputs are then ready by ~14k and u1 lands before px2 needs it;
        # the reload still finishes well before the first retrieval silu
        ev_p1h0 = nc.scalar.activation(
            g_tm[1][:, 0:4].rearrange("p c d -> p (c d)"), pg1[0][:], AF.Copy
        )
        _dep(ev_p1h0.ins, dsilu_insts[-1].ins, sync=False,
             reason="ACT order: dsilus before pg1 evicts")
        ev_p1h1 = nc.scalar.activation(
            g_tm[1][:, 4:8].rearrange("p c d -> p (c d)"), pg1[1][:], AF.Copy
        )
        _dep(ev_p1h1.ins, ev_p1h0.ins, sync=False, reason="ACT order")
        _dep(dummy.ins, ev_p1h1.ins, sync=False,
             reason="ACT order: pg1 evicts before reload")

        # ---- M = S^T G0 in its OWN bank/group so m_r never waits the
        # late dW1 round-trip ----
        macc = pstage("macc", w=128)
        m_stop = None
        for k, c in enumerate((0, 1, 2, 3, 4, 5, 6, 7)):
            m_stop = nc.tensor.matmul(
                macc[:, 0:128],
                s_tmb[:, c],
                g_tm[0][:, c],
                start=(k == 0),
                stop=(c == 7),
            )
            if k == 0:
                _dep(m_stop.ins, last_pg0.ins, sync=False,
                     reason="PE order: M right after g0 transposes")
        m_r = big.tile([128, 128], bf16, tag="m_r")
        nc.vector.tensor_copy(m_r[:], macc[:, 0:128])

        # ---- dW1 in its OWN bank/group: u1 feeds retrieval layer 2 and
        # must not wait behind dW3/dW2's group stop ----
        dw1acc = pstage("dw1acc", w=128)
        dw1_stop = None
        dw1_first = None
        for k, c in enumerate((0, 1, 2, 3, 4, 5, 6, 7)):
            dw1_stop = nc.tensor.matmul(
                dw1acc[:, 0:128],
                a_tm[1][:, c],
                g_tm[1][:, c],
                start=(k == 0),
                stop=(c == 7),
            )
            if k == 0:
                dw1_first = dw1_stop
                _dep(dw1_first.ins, last_cmm.ins, sync=False,
                     reason="PE order: backward before dW")
                _dep(dw1_first.ins, m_stop.ins, sync=False,
                     reason="PE order: M before dW1")

        # ---- dW2 in hb1 (free after c0t1's mul): u2 lands right
        # after X1stop instead of waiting dW3's group ----
        dw2acc = pbank("hb1", "dw2acc", shape=[128, 128])
        dw2_stop = None
        dw2_first = None
        for c in range(NCHUNK):
            dw2_stop = nc.tensor.matmul(
                dw2acc[:],
                a_tm[2][:, c],
                g_tm[2][:, c],
                start=(c == 0),
                stop=(c == NCHUNK - 1),
            )
            if c == 0:
                dw2_first = dw2_stop
                _dep(dw2_first.ins, last_cmm.ins, sync=False,
                     reason="PE order: backward before dW")
                _dep(dw2_first.ins, m_stop.ins, sync=False,
                     reason="PE order: M before dW2")

        # ---- dW3 group (only feeds u3, needed last) ----
        acc = pbank("hc1", "dwacc", shape=[128, 128])
        dw_stop = None
        dw_first = None
        for c in range(NCHUNK):
            dw_stop = nc.tensor.matmul(
                acc[:],
                a_tm[3][:, c],
                g_tm[3][:, c],
                start=(c == 0),
                stop=(c == NCHUNK - 1),
            )
            if dw_first is None:
                dw_first = dw_stop
                _dep(dw_first.ins, last_cmm.ins, sync=False,
                     reason="PE order: backward before dW")
                _dep(dw_first.ins, m_stop.ins, sync=False,
                     reason="PE order: M before dW3")

        u = [None]
        for nm, accb, stop_i, wf in (
            ("u1", dw1acc[:, 0:128], dw1_stop, w_f[0]),
            ("u2", dw2acc[:], dw2_stop, w_f[1]),
            ("u3", acc[:], dw_stop, w_f[2]),
        ):
            ut = big.tile([D, D], bf16, name=nm, tag=nm)
            ai = nc.vector.tensor_add(ut[:], accb, wf)
            # same-bank safety: no reads before the group's stop matmul
            _dep(ai.ins, stop_i.ins, sync=True, reason=f"{nm} bank group")
            u.append(ut)
        # u[1]=w1+dW1, u[2]=w2+dW2, u[3]=w3+dW3

        # ---- retrieval: X1 = (wq w0)^T S^T + M^T P, then layers 2..4 ----
        r1, r2, r3 = [], [], []
        for h in range(NTI):
            r1.append(big.tile([128, RH], bf16, name=f"r1h{h}", tag=f"r1h{h}"))
            r2.append(big.tile([128, RH], bf16, name=f"r2h{h}", tag=f"r2h{h}"))
            r3.append(big.tile([128, RH], bf16, name=f"r3h{h}", tag=f"r3h{h}"))

        nh = NT // RH
        px1 = [pbank(f"ha{hh}", f"px1_{hh}", shape=[128, RH]) for hh in range(nh)]
        for hh in range(nh):
            # term 1 ((wq w0)^T S^T) has no M dependency -- runs during dW
            t1mm = nc.tensor.matmul(
                px1[hh][:], w0q, stt[0][:, ts(hh, RH)], start=True, stop=False
            )
            # px1's bank frees after the sp0-t0 dsilu; run term1 in the
            # PE idle gaps of the backward-mul phase
            _dep(t1mm.ins, last_cmm.ins, sync=False,
                 reason="PE order: backward mms before px1 term1")
        for hh in range(nh):
            smm = nc.tensor.matmul(
                px1[hh][:], m_r[:], pt[:, ts(hh, RH)], start=False, stop=True
            )
            # dW1 fills the PE window while m_r evicts on DVE; the X1
            # stops come right after it, and dW2/dW3 stay behind them
            _dep(smm.ins, dw1_stop.ins, sync=False,
                 reason="PE order: dW1 before X1 stop")
            _dep(dw_first.ins, smm.ins, sync=False,
                 reason="PE order: X1 stop before dW3")
            _dep(dw2_first.ins, smm.ins, sync=False,
                 reason="PE order: X1 stop before dW2")
            nc.scalar.activation(r1[hh][:], px1[hh][:], AF.Silu)
        px2 = [pbank(f"hb{hh}", f"px2_{hh}", shape=[128, RH]) for hh in range(nh)]
        for hh in range(nh):
            pmm = nc.tensor.matmul(px2[hh][:], u[1][:], r1[hh][:])
            _dep(dw_first.ins, pmm.ins, sync=False,
                 reason="PE order: px2 before dW3")
            nc.scalar.activation(r2[hh][:], px2[hh][:], AF.Silu)
        px3 = [
            pbank("hc0", "px3_0", shape=[128, RH]),
            pstage("px3_1", w=RH),
        ]
        for hh in range(nh):
            nc.tensor.matmul(px3[hh][:], u[2][:], r2[hh][:])
            nc.scalar.activation(r3[hh][:], px3[hh][:], AF.Silu)
        out_r = out_dr  # [p, c, d]: token c*128+p, contiguous per partition
        o_tm = big.tile([128, 4, 128], bf16, tag="o_tm")
        for hh in range(nh):
            po = pstage(f"po{hh}", w=RH)
            pov = po[:].rearrange("p (c d) -> p c d", d=128)
            for j in range(RH // 128):
                nc.tensor.matmul(
                    pov[:, j], r3[hh][:, ts(j, 128)], u[3][:],
                    start=(j == 0), stop=(j == RH // 128 - 1),
                )
            if hh == 0:
                nc.vector.tensor_copy(o_tm[:, 0:2], pov[:])
            else:
                # ACT frees right after its last retrieval silu -- evicting
                # half 1 there runs both halves' drains in parallel
                nc.scalar.activation(
                    o_tm[:, 2:4].rearrange("p c d -> p (c d)"),
                    pov[:].rearrange("p c d -> p (c d)"), AF.Copy
                )
        # out DMA split across two queues (SP + ACT) so both halves'
        # dispatch/descriptor-gen/latency pipelines run in parallel --
        # the h0 DMA is fully hidden under h1's eviction
        nc.sync.dma_start(out_r[:, 0:2], o_tm[:, 0:2])
        nc.scalar.dma_start(out_r[:, 2:4], o_tm[:, 2:4])


_CACHE = {}


def _get_nc():
    if "nc" not in _CACHE:
        _CACHE["nc"] = _build_program()
    return _CACHE["nc"]


def _bf(x):
    return np.ascontiguousarray(x.astype(ml_dtypes.bfloat16))


def _prep_weights(w0, w1, w2, w3, wq, wkv):
    """Host-side weight-space prep (layout, transposes, scales, composes)."""
    w0, w1, w2, w3, wq, wkv = (
        np.asarray(x, np.float32) for x in (w0, w1, w2, w3, wq, wkv)
    )
    wk, wv = wkv[:, :D], wkv[:, D:]
    ident = np.eye(D, dtype=np.float32)
    w0eff = wk @ w0
    wpbu = np.concatenate(
        [
            w1, w2,
            (2.0 / D) * w3,     # w3s
            (-2.0 / D) * wv,    # wv_r
            wq @ w0,            # w0q: X1 term1 = (wq w0)^T S^T
            wq @ wk.T,          # wkq_t: pt = (wq Wk^T)^T S^T
        ],
        axis=1,
    )
    wpbr = np.concatenate([w1.T, w2.T, w3.T, ident], axis=1)
    wpf = np.ascontiguousarray(np.concatenate([w1, w2, w3], axis=1))
    return _bf(w0eff), _bf(wpbu), _bf(wpbr), wpf


def kernel(seq, w0, w1, w2, w3, wq, wkv):
    nc = _get_nc()
    seq = np.asarray(seq, np.float32)
    w0eff, wpbu, wpbr, wpf = _prep_weights(w0, w1, w2, w3, wq, wkv)

    in_maps = []
    for c in range(NCORES):
        b, h = c // 2, c % 2
        if h == 0:
            s = seq[b]
        else:
            # rotate: retrieval half first; grad sum is order-invariant
            s = np.concatenate([seq[b, NT:], seq[b, :NT]], axis=0)
        sb = s.astype(ml_dtypes.bfloat16)
        # stp = [w0eff | S^T] so the first DMA carries layer-0's weight
        stp = np.ascontiguousarray(
            np.concatenate([w0eff, np.ascontiguousarray(sb.T)], axis=1)
        )
        # token-major [128, c, d] flattened: partition p, token c*128+p
        stm = np.ascontiguousarray(
            sb.reshape(NCHUNK, 128, D).transpose(1, 0, 2).reshape(128, N)
        )
        in_maps.append(
            {
                "stp": stp,
                "s_tmb": stm,
                "wpbu": wpbu,
                "wpbr": wpbr,
                "wpf": wpf,
            }
        )

    res = run_bass_kernel_spmd(nc, in_maps, core_ids=list(range(NCORES)))
    _CACHE["last_results"] = res

    out = np.empty((B, N, D), np.float32)
    for c in range(NCORES):
        b, h = c // 2, c % 2
        # device layout [p, chunk, d] -> tokens (chunk*128+p, d)
        ob = res.results[c]["out"].astype(np.float32)
        out[b, h * NT : (h + 1) * NT] = ob.transpose(1, 0, 2).reshape(NT, D)
    return out


# revision 35
# speedup vs baseline: 1.0151x; 1.0151x over previous
"""Trainium2 Bass kernel for nn_NeuralMemory (B=4, N=1024, D=128, DEPTH=4).

Sharding: 8 cores, core c handles batch b = c//2. Both cores of a pair
compute the store phase (per-token grads summed over all 1024 tokens)
redundantly -- the grad sum is order-invariant, so each core gets its
batch's sequence with its own retrieval half rotated to the front and
retrieves tokens [0:512) of its view. No collectives (a pair AllReduce
has a ~10us floor, worse than the duplicated compute).

v5 design notes (on top of v4):
  - Input DMA head: w0eff is prepended to S^T host-side and shipped as
    DMA1 = [w0eff | S^T tokens 0:512) (160KB) so layer 0 + the whole
    ACT chain starts at the first notify (~3.35us); DMA2 = the t1 half.
  - x0 is gone: host composes w0q = wq @ w0 so retrieval layer-1 term 1
    is a single matmul straight off S^T (saves a mm + a DVE evict).
  - Engine rebalance for the backward-mul drain (the old g muls were 6
    serialized 658ns DVE ops, landing g0 ~2us after the last dsilu):
    t1-half muls + both pg1 evicts run on the otherwise-idle Pool
    engine; DVE keeps the M-critical t0 chain, pg0 evicts, m_r, the
    u adds and the output evicts. ACT does only silu/dsilu/reload +
    retrieval silus.
  - All v4 disciplines kept: bf16 everywhere, per-half tiles for
    tile-granular deps, PSUM bank reuse plan, XBAR for a1..a3/g3/g2
    token-major copies, PE-transpose + evict for g0/g1, M/dW1 in their
    own PSUM groups, ACT-table order silu -> dsilu -> silu.
"""

import numpy as np
import ml_dtypes

import concourse.bass as bass
import concourse.mybir as mybir
import concourse.tile as tile
from concourse import bacc
from concourse.bass import ts
from concourse.bass_utils import run_bass_kernel_spmd

B, N, D = 4, 1024, 128
NCORES = 8
NT = 512            # tokens retrieved per core (half a batch)
TT = 512            # store-phase token tile
NTI = N // TT
NCHUNK = N // 128
RH = 256            # retrieval sub-tile

# bf16 weight packs:
#  wpbu (urgent): w1 | w2 | w3s=(2/D)w3 | wv_r=-(2/D)Wv | w0q=wq@w0
#                 | wkq_t=wq@Wk^T
#  wpbr (rest):   w1^T | w2^T | w3^T | ident
#  stp:           [w0eff=Wk@w0 | S^T]  (split into two DMAs, t0 first)

f32 = mybir.dt.float32
bf16 = mybir.dt.bfloat16

AF = mybir.ActivationFunctionType
ALU = mybir.AluOpType


def _build_program(reps=1):
    nc = bacc.Bacc(
        "TRN2",
        target_bir_lowering=False,
        debug=False,
        enable_asserts=False,
        num_devices=NCORES,
    )

    stp_dr = nc.dram_tensor("stp", [128, D + N], bf16, kind="ExternalInput").ap()
    stm_dr = nc.dram_tensor("s_tmb", [128, N], bf16, kind="ExternalInput").ap()
    wbu_dr = nc.dram_tensor("wpbu", [D, 6 * D], bf16, kind="ExternalInput").ap()
    wbr_dr = nc.dram_tensor("wpbr", [D, 4 * D], bf16, kind="ExternalInput").ap()
    wf_dr = nc.dram_tensor("wpf", [D, 3 * D], f32, kind="ExternalInput").ap()
    out_dr = nc.dram_tensor("out", [128, NT // 128, D], bf16, kind="ExternalOutput").ap()

    with tile.TileContext(nc) as tc:
        for _ in range(reps):
            _emit(tc, stp_dr, stm_dr, wbu_dr, wbr_dr, wf_dr, out_dr)

    nc.compile()
    return nc


def _emit(tc, stp_dr, stm_dr, wbu_dr, wbr_dr, wf_dr, out_dr):
    nc = tc.nc
    from contextlib import ExitStack

    from concourse.tile_rust import add_dep_helper as _dep  # type: ignore

    with ExitStack() as ctx:
        consts = ctx.enter_context(tc.tile_pool(name="consts", bufs=1))
        big = ctx.enter_context(tc.tile_pool(name="big", bufs=1))
        # PSUM: ha0,ha1,hb0,hb1,hc0,hc1 (1 bank each) + stg(2) = 8 banks
        pp = ctx.enter_context(tc.tile_pool(name="pp", bufs=1, space="PSUM"))

        def pbank(tag, name, shape=None, dt=f32):
            return pp.tile(shape or [128, TT], dt, tag=tag, bufs=1, name=name)

        def pstage(name, w=512, dt=f32):
            return pp.tile([128, w], dt, tag="stg", bufs=2, name=name)

        # ---- DMAs, ordered by need (all HWDGE on the sync queue);
        # DMA1 = [w0eff | S^T t0-half] gates the whole forward chain ----
        sta = big.tile([128, D + TT], bf16, tag="sta")   # w0eff | S^T[:, :512]
        stb = big.tile([128, TT], bf16, tag="stb")       # S^T[:, 512:]
        wpbu = consts.tile([D, 6 * D], bf16, tag="wpbu")
        wpbr = consts.tile([D, 4 * D], bf16, tag="wpbr")
        s_tmb = big.tile([128, NCHUNK, 128], bf16, tag="s_tmb")  # token-major
        wpf = consts.tile([D, 3 * D], f32, tag="wpf")
        nc.sync.dma_start(sta[:], stp_dr[:, 0 : D + TT])
        nc.sync.dma_start(stb[:], stp_dr[:, D + TT :])
        nc.sync.dma_start(wpbu[:], wbu_dr)
        nc.sync.dma_start(wpbr[:], wbr_dr)
        nc.sync.dma_start(
            s_tmb[:], stm_dr.rearrange("p (c d) -> p c d", d=128)
        )
        nc.sync.dma_start(wpf[:], wf_dr)

        w0eff = sta[:, 0:D]
        stt = [sta[:, D : D + TT], stb[:]]
        w1b = wpbu[:, ts(0, D)]
        w2b = wpbu[:, ts(1, D)]
        w3s = wpbu[:, ts(2, D)]
        wv_r = wpbu[:, ts(3, D)]
        w0q = wpbu[:, ts(4, D)]                        # wq @ w0
        wkq_t = wpbu[:, ts(5, D)]                      # wq @ Wk^T
        wt = [wpbr[:, ts(i, D)] for i in range(3)]     # w1^T,w2^T,w3^T
        ident_b = wpbr[:, ts(3, D)]
        w_f = [wpf[:, ts(i, D)] for i in range(3)]     # w1,w2,w3 f32

        # tiny scratch silu pulls the first ACT table load off the
        # critical path (runs during the DMAs)
        scr = consts.tile([128, 1], f32, tag="scr")
        scr2 = consts.tile([128, 1], f32, tag="scr2")
        nc.gpsimd.memset(scr[:], 0.0)
        nc.scalar.activation(scr2[:], scr[:], AF.Silu)

        # PE warm-up: start the HAM clock window early so real matmuls
        # hit full clock by ~3us
        wupa = consts.tile([128, 128], f32, tag="wupa")
        nc.gpsimd.memset(wupa[:], 0.0)
        wupp = pstage("wupp")
        for _ in range(4):
            nc.tensor.matmul(
                wupp[:, 0:128], wupa[:], wupa[:],
                skip_group_check=True,
            )

        # per-half persistent SBUF tensors (feature-major, bf16)
        def halves(pfx):
            return [
                big.tile([128, TT], bf16, name=f"{pfx}{t}", tag=f"{pfx}{t}")
                for t in range(NTI)
            ]

        a1 = halves("a1")
        a2 = halves("a2")
        a3 = halves("a3")
        sp0 = halves("sp0")
        sp1 = halves("sp1")
        sp2 = halves("sp2")
        g0 = halves("g0")
        g1 = halves("g1")
        g2 = halves("g2")
        g3 = halves("g3")   # (2/D)(H3 - V)

        silu_insts = []
        dsilu_insts = []

        # ---- forward: six single-bank H tiles; silu -> bf16 halves ----
        hb = {}
        for li, tag in ((0, "ha"), (1, "hb"), (2, "hc")):
            for t in range(NTI):
                hb[li, t] = pbank(f"{tag}{t}", f"h{li}_{t}")
        pt = big.tile([128, NT], bf16, tag="pt")
        l2mm = {}
        for t in range(NTI):
            nc.tensor.matmul(hb[0, t][:], w0eff, stt[t])
            silu_insts.append(nc.scalar.activation(a1[t][:], hb[0, t][:], AF.Silu))
            l2mm[t] = nc.tensor.matmul(hb[1, t][:], w1b, a1[t][:])
            silu_insts.append(nc.scalar.activation(a2[t][:], hb[1, t][:], AF.Silu))
            nc.tensor.matmul(hb[2, t][:], w2b, a2[t][:])
            silu_insts.append(nc.scalar.activation(a3[t][:], hb[2, t][:], AF.Silu))
            # H3 - V accumulated in one stage bank (wv_r is negated+scaled)
            h3 = pstage(f"h3_{t}")
            nc.tensor.matmul(h3[:], wv_r, stt[t], start=True, stop=False)
            nc.tensor.matmul(h3[:], w3s, a3[t][:], start=False, stop=True)
            nc.vector.tensor_copy(g3[t][:], h3[:])
            if t == 0:
                # pt = (wq Wk^T)^T S^T: independent matmul off S^T t0;
                # pinned after L1-t0 so it never sits in the forward's
                # critical PE slot (it's only needed by the backward-era
                # X1 stop matmuls)
                px = pstage("p_pt")
                ptmm = nc.tensor.matmul(px[:], wkq_t, stt[0])
                _dep(ptmm.ins, l2mm[0].ins, sync=False,
                     reason="PE order: forward L1-t0 before pt")
                nc.vector.tensor_copy(pt[:], px[:])

        # ---- dsilu after all silus (one table switch); t0 first so the
        # backward c-mm/mul chain (same order) never queues behind the
        # other half's dsilu ----
        for spd, li in ((sp2, 2), (sp1, 1), (sp0, 0)):
            for t in (0, 1):
                di = nc.scalar.activation(
                    spd[t][:], hb[li, t][:], AF.Derivative_silu
                )
                dsilu_insts.append(di)
        for di in dsilu_insts:
            _dep(di.ins, silu_insts[-1].ins, sync=False, reason="act-table order")

        # ---- backward: tile-1 first (it gates M); c tiles reuse the H
        # banks (freed in dsilu order, which matches mul consumption).
        # Engine split: the t1-half muls go to the idle Pool engine so
        # the M-critical t0 chain never queues on DVE. ----
        # c-bank plan: c2 -> the stage banks (free right after the g3
        # evicts, so the first backward mm never waits a dsilu WAR),
        # c1 -> h2's banks (free after the sp2 dsilus), c0 -> h1's banks.
        cb = {}
        for t in (0, 1):
            cb[2, t] = pstage(f"c2_{t}")
        for li, tag in ((1, "hc"), (0, "hb")):
            for t in (0, 1):
                cb[li, t] = pbank(f"{tag}{t}", f"c{li}_{t}")
        last_cmm = None
        for li, gin, gout, spd in (
            (2, g3, g2, sp2), (1, g2, g1, sp1), (0, g1, g0, sp0)
        ):
            for t in (0, 1):
                last_cmm = nc.tensor.matmul(cb[li, t][:], wt[li], gin[t][:])
                nc.vector.tensor_mul(gout[t][:], cb[li, t][:], spd[t][:])

        # reload the silu table during the dW phase, off the tail
        scr3 = consts.tile([128, 1], f32, tag="scr3")
        dummy = nc.scalar.activation(scr3[:], scr[:], AF.Silu)
        _dep(dummy.ins, dsilu_insts[-1].ins, sync=False, reason="act-table order")

        # ---- token-major copies: XBAR for a1..a3,g3,g2; PE path for g0/g1 --
        a_tm = [None] + [
            big.tile([128, NCHUNK, 128], bf16, name=f"atm{i}", tag=f"atm{i}")
            for i in (1, 2, 3)
        ]
        g_tm = [
            big.tile([128, NCHUNK, 128], bf16, name=f"gtm{i}", tag=f"gtm{i}")
            for i in range(4)
        ]
        for src, dst, horder in (
            (a1, a_tm[1], (0, 1)), (a2, a_tm[2], (0, 1)), (a3, a_tm[3], (0, 1)),
            (g3, g_tm[3], (0, 1)), (g2, g_tm[2], (0, 1)),
        ):
            for h in horder:
                nc.sync.dma_start_transpose(dst[:, 4 * h : 4 * h + 4], src[h][:])

        # g1 feeds dW1 -> u1 -> retrieval layer 2; PE transposes into the
        # freed hb banks, both halves evicted on Pool (order-pinned after
        # the Pool g0t1 mul so M is never delayed).
        pg1 = [None, None]
        for h in (0, 1):
            pg1[h] = pbank(f"hc{h}", f"pg1_{h}", shape=[128, TT], dt=bf16)
            for j in range(4):
                nc.tensor.matmul(
                    pg1[h][:, ts(j, 128)], g1[h][:, ts(j, 128)], ident_b,
                    is_transpose=True,
                )

        # g0: PE transposes + DVE half evicts (lowest latency on the
        # tail); separate stage tiles per half so the t1 transposes don't
        # wait on the t0 eviction (tile-granular WAR). t0's mul lands
        # first, so t0 transposes/evicts/M-chunks all go first.
        pg0 = [pstage(f"p_g0{h}", w=512, dt=bf16) for h in range(NTI)]
        last_pg0 = None
        for h in (0, 1):
            for j in range(4):
                last_pg0 = nc.tensor.matmul(
                    pg0[h][:, ts(j, 128)], g0[h][:, ts(j, 128)], ident_b,
                    is_transpose=True,
                )
        ev_g0h0 = nc.vector.tensor_copy(
            g_tm[0][:, 0:4], pg0[0][:].rearrange("p (c d) -> p c d", d=128)
        )
        ev_g0h1 = nc.vector.tensor_copy(
            g_tm[0][:, 4:8], pg0[1][:].rearrange("p (c d) -> p c d", d=128)
        )
        _dep(ev_g0h1.ins, ev_g0h0.ins, sync=False, reason="DVE order: t0 evict first")

        # pg1 evicts on ACT right after the dsilus and BEFORE the silu
        # reload (Copy is in every table, so no extra load) -- dW1's
        # inputs are then ready by ~14k and u1 lands before px2 needs it;
        # the reload still finishes well before the first retrieval silu
        ev_p1h0 = nc.scalar.activation(
            g_tm[1][:, 0:4].rearrange("p c d -> p (c d)"), pg1[0][:], AF.Copy
        )
        _dep(ev_p1h0.ins, dsilu_insts[-1].ins, sync=False,
             reason="ACT order: dsilus before pg1 evicts")
        ev_p1h1 = nc.scalar.activation(
            g_tm[1][:, 4:8].rearrange("p c d -> p (c d)"), pg1[1][:], AF.Copy
        )
        _dep(ev_p1h1.ins, ev_p1h0.ins, sync=False, reason="ACT order")
        _dep(dummy.ins, ev_p1h1.ins, sync=False,
             reason="ACT order: pg1 evicts before reload")

        # ---- M = S^T G0 in its OWN bank/group so m_r never waits the
        # late dW1 round-trip ----
        macc = pstage("macc", w=128)
        m_stop = None
        for k, c in enumerate((0, 1, 2, 3, 4, 5, 6, 7)):
            m_stop = nc.tensor.matmul(
                macc[:, 0:128],
                s_tmb[:, c],
                g_tm[0][:, c],
                start=(k == 0),
                stop=(c == 7),
            )
            if k == 0:
                _dep(m_stop.ins, last_pg0.ins, sync=False,
                     reason="PE order: M right after g0 transposes")
        m_r = big.tile([128, 128], bf16, tag="m_r")
        nc.vector.tensor_copy(m_r[:], macc[:, 0:128])

        # ---- dW1 in its OWN bank/group: u1 feeds retrieval layer 2 and
        # must not wait behind dW3/dW2's group stop ----
        dw1acc = pstage("dw1acc", w=128)
        dw1_stop = None
        dw1_first = None
        for k, c in enumerate((0, 1, 2, 3, 4, 5, 6, 7)):
            dw1_stop = nc.tensor.matmul(
                dw1acc[:, 0:128],
                a_tm[1][:, c],
                g_tm[1][:, c],
                start=(k == 0),
                stop=(c == 7),
            )
            if k == 0:
                dw1_first = dw1_stop
                _dep(dw1_first.ins, last_cmm.ins, sync=False,
                     reason="PE order: backward before dW")
                _dep(dw1_first.ins, m_stop.ins, sync=False,
                     reason="PE order: M before dW1")

        # ---- dW2 in hb1 (free after c0t1's mul): u2 lands right
        # after X1stop instead of waiting dW3's group ----
        dw2acc = pbank("hb1", "dw2acc", shape=[128, 128])
        dw2_stop = None
        dw2_first = None
        for c in range(NCHUNK):
            dw2_stop = nc.tensor.matmul(
                dw2acc[:],
                a_tm[2][:, c],
                g_tm[2][:, c],
                start=(c == 0),
                stop=(c == NCHUNK - 1),
            )
            if c == 0:
                dw2_first = dw2_stop
                _dep(dw2_first.ins, last_cmm.ins, sync=False,
                     reason="PE order: backward before dW")
                _dep(dw2_first.ins, m_stop.ins, sync=False,
                     reason="PE order: M before dW2")

        # ---- dW3 group (only feeds u3, needed last) ----
        acc = pbank("hc1", "dwacc", shape=[128, 128])
        dw_stop = None
        dw_first = None
        for c in range(NCHUNK):
            dw_stop = nc.tensor.matmul(
                acc[:],
                a_tm[3][:, c],
                g_tm[3][:, c],
                start=(c == 0),
                stop=(c == NCHUNK - 1),
            )
            if dw_first is None:
                dw_first = dw_stop
                _dep(dw_first.ins, last_cmm.ins, sync=False,
                     reason="PE order: backward before dW")
                _dep(dw_first.ins, m_stop.ins, sync=False,
                     reason="PE order: M before dW3")

        u = [None]
        for nm, accb, stop_i, wf in (
            ("u1", dw1acc[:, 0:128], dw1_stop, w_f[0]),
            ("u2", dw2acc[:], dw2_stop, w_f[1]),
            ("u3", acc[:], dw_stop, w_f[2]),
        ):
            ut = big.tile([D, D], bf16, name=nm, tag=nm)
            ai = nc.vector.tensor_add(ut[:], accb, wf)
            # same-bank safety: no reads before the group's stop matmul
            _dep(ai.ins, stop_i.ins, sync=True, reason=f"{nm} bank group")
            u.append(ut)
        # u[1]=w1+dW1, u[2]=w2+dW2, u[3]=w3+dW3

        # ---- retrieval: X1 = (wq w0)^T S^T + M^T P, then layers 2..4 ----
        # asymmetric retrieval halves: h0 = 384 tokens, h1 = 128. Total
        # ACT work is unchanged, but the LAST half's silu/po/evict (the
        # tail gate) all shrink by 3x.
        RHS = (384, 128)
        ROF = (0, 384)
        nh = 2
        r1, r2, r3 = [], [], []
        for h in range(nh):
            r1.append(big.tile([128, RHS[h]], bf16, name=f"r1h{h}", tag=f"r1h{h}"))
            r2.append(big.tile([128, RHS[h]], bf16, name=f"r2h{h}", tag=f"r2h{h}"))
            r3.append(big.tile([128, RHS[h]], bf16, name=f"r3h{h}", tag=f"r3h{h}"))

        px1 = [pbank(f"ha{hh}", f"px1_{hh}", shape=[128, RHS[hh]]) for hh in range(nh)]
        for hh in range(nh):
            # term 1 ((wq w0)^T S^T) has no M dependency -- runs during dW
            t1mm = nc.tensor.matmul(
                px1[hh][:], w0q, stt[0][:, ROF[hh] : ROF[hh] + RHS[hh]],
                start=True, stop=False
            )
            # px1's bank frees after the sp0-t0 dsilu; run term1 in the
            # PE idle gaps of the backward-mul phase
            _dep(t1mm.ins, last_cmm.ins, sync=False,
                 reason="PE order: backward mms before px1 term1")
        for hh in range(nh):
            smm = nc.tensor.matmul(
                px1[hh][:], m_r[:], pt[:, ROF[hh] : ROF[hh] + RHS[hh]],
                start=False, stop=True
            )
            # dW1 fills the PE window while m_r evicts on DVE; the X1
            # stops come right after it, and dW2/dW3 stay behind them
            _dep(smm.ins, dw1_stop.ins, sync=False,
                 reason="PE order: dW1 before X1 stop")
            _dep(dw_first.ins, smm.ins, sync=False,
                 reason="PE order: X1 stop before dW3")
            _dep(dw2_first.ins, smm.ins, sync=False,
                 reason="PE order: X1 stop before dW2")
            nc.scalar.activation(r1[hh][:], px1[hh][:], AF.Silu)
        px2 = [pbank(f"hb{hh}", f"px2_{hh}", shape=[128, RHS[hh]]) for hh in range(nh)]
        for hh in range(nh):
            pmm = nc.tensor.matmul(px2[hh][:], u[1][:], r1[hh][:])
            _dep(dw_first.ins, pmm.ins, sync=False,
                 reason="PE order: px2 before dW3")
            nc.scalar.activation(r2[hh][:], px2[hh][:], AF.Silu)
        px3 = [
            pbank("hc0", "px3_0", shape=[128, RHS[0]]),
            pstage("px3_1", w=RHS[1]),
        ]
        for hh in range(nh):
            nc.tensor.matmul(px3[hh][:], u[2][:], r2[hh][:])
            nc.scalar.activation(r3[hh][:], px3[hh][:], AF.Silu)
        out_r = out_dr  # [p, c, d]: token c*128+p, contiguous per partition
        o_tm = big.tile([128, 4, 128], bf16, tag="o_tm")
        for hh in range(nh):
            po = pstage(f"po{hh}", w=RHS[hh])
            pov = po[:].rearrange("p (c d) -> p c d", d=128)
            nchk = RHS[hh] // 128
            for j in range(nchk):
                nc.tensor.matmul(
                    pov[:, j], r3[hh][:, ts(j, 128)], u[3][:],
                    start=(j == 0), stop=(j == nchk - 1),
                )
            if hh == 0:
                # ACT is free once r3h1's silu retires and h0's po matmuls
                # are long done -- evicting h0 there runs in parallel with
                # DVE's h1 evict
                nc.scalar.activation(
                    o_tm[:, 0:3].rearrange("p c d -> p (c d)"),
                    pov[:].rearrange("p c d -> p (c d)"), AF.Copy
                )
            else:
                nc.vector.tensor_copy(o_tm[:, 3:4], pov[:])
        # single out DMA: one HWDGE gen + one completion-sem wait on the
        # tail (splitting pays a second serial HWDGE gen + sem -- worse)
        nc.sync.dma_start(out_r[:], o_tm[:])


_CACHE = {}


def _get_nc():
    if "nc" not in _CACHE:
        _CACHE["nc"] = _build_program()
    return _CACHE["nc"]


def _bf(x):
    return np.ascontiguousarray(x.astype(ml_dtypes.bfloat16))


def _prep_weights(w0, w1, w2, w3, wq, wkv):
    """Host-side weight-space prep (layout, transposes, scales, composes)."""
    w0, w1, w2, w3, wq, wkv = (
        np.asarray(x, np.float32) for x in (w0, w1, w2, w3, wq, wkv)
    )
    wk, wv = wkv[:, :D], wkv[:, D:]
    ident = np.eye(D, dtype=np.float32)
    w0eff = wk @ w0
    wpbu = np.concatenate(
        [
            w1, w2,
            (2.0 / D) * w3,     # w3s
            (-2.0 / D) * wv,    # wv_r
            wq @ w0,            # w0q: X1 term1 = (wq w0)^T S^T
            wq @ wk.T,          # wkq_t: pt = (wq Wk^T)^T S^T
        ],
        axis=1,
    )
    wpbr = np.concatenate([w1.T, w2.T, w3.T, ident], axis=1)
    wpf = np.ascontiguousarray(np.concatenate([w1, w2, w3], axis=1))
    return _bf(w0eff), _bf(wpbu), _bf(wpbr), wpf


def kernel(seq, w0, w1, w2, w3, wq, wkv):
    nc = _get_nc()
    seq = np.asarray(seq, np.float32)
    w0eff, wpbu, wpbr, wpf = _prep_weights(w0, w1, w2, w3, wq, wkv)

    in_maps = []
    for c in range(NCORES):
        b, h = c // 2, c % 2
        if h == 0:
            s = seq[b]
        else:
            # rotate: retrieval half first; grad sum is order-invariant
            s = np.concatenate([seq[b, NT:], seq[b, :NT]], axis=0)
        sb = s.astype(ml_dtypes.bfloat16)
        # stp = [w0eff | S^T] so the first DMA carries layer-0's weight
        stp = np.ascontiguousarray(
            np.concatenate([w0eff, np.ascontiguousarray(sb.T)], axis=1)
        )
        # token-major [128, c, d] flattened: partition p, token c*128+p
        stm = np.ascontiguousarray(
            sb.reshape(NCHUNK, 128, D).transpose(1, 0, 2).reshape(128, N)
        )
        in_maps.append(
            {
                "stp": stp,
                "s_tmb": stm,
                "wpbu": wpbu,
                "wpbr": wpbr,
                "wpf": wpf,
            }
        )

    res = run_bass_kernel_spmd(nc, in_maps, core_ids=list(range(NCORES)))
    _CACHE["last_results"] = res

    out = np.empty((B, N, D), np.float32)
    for c in range(NCORES):
        b, h = c // 2, c % 2
        # device layout [p, chunk, d] -> tokens (chunk*128+p, d)
        ob = res.results[c]["out"].astype(np.float32)
        out[b, h * NT : (h + 1) * NT] = ob.transpose(1, 0, 2).reshape(NT, D)
    return out


# revision 37
# speedup vs baseline: 1.0160x; 1.0008x over previous
"""Trainium2 Bass kernel for nn_NeuralMemory (B=4, N=1024, D=128, DEPTH=4).

Sharding: 8 cores, core c handles batch b = c//2. Both cores of a pair
compute the store phase (per-token grads summed over all 1024 tokens)
redundantly -- the grad sum is order-invariant, so each core gets its
batch's sequence with its own retrieval half rotated to the front and
retrieves tokens [0:512) of its view. No collectives (a pair AllReduce
has a ~10us floor, worse than the duplicated compute).

v5 design notes (on top of v4):
  - Input DMA head: w0eff is prepended to S^T host-side and shipped as
    DMA1 = [w0eff | S^T tokens 0:512) (160KB) so layer 0 + the whole
    ACT chain starts at the first notify (~3.35us); DMA2 = the t1 half.
  - x0 is gone: host composes w0q = wq @ w0 so retrieval layer-1 term 1
    is a single matmul straight off S^T (saves a mm + a DVE evict).
  - Engine rebalance for the backward-mul drain (the old g muls were 6
    serialized 658ns DVE ops, landing g0 ~2us after the last dsilu):
    t1-half muls + both pg1 evicts run on the otherwise-idle Pool
    engine; DVE keeps the M-critical t0 chain, pg0 evicts, m_r, the
    u adds and the output evicts. ACT does only silu/dsilu/reload +
    retrieval silus.
  - All v4 disciplines kept: bf16 everywhere, per-half tiles for
    tile-granular deps, PSUM bank reuse plan, XBAR for a1..a3/g3/g2
    token-major copies, PE-transpose + evict for g0/g1, M/dW1 in their
    own PSUM groups, ACT-table order silu -> dsilu -> silu.
"""

import numpy as np
import ml_dtypes

import concourse.bass as bass
import concourse.mybir as mybir
import concourse.tile as tile
from concourse import bacc
from concourse.bass import ts
from concourse.bass_utils import run_bass_kernel_spmd

B, N, D = 4, 1024, 128
NCORES = 8
NT = 512            # tokens retrieved per core (half a batch)
TT = 512            # store-phase token tile
NTI = N // TT
NCHUNK = N // 128
RH = 256            # retrieval sub-tile

# bf16 weight packs:
#  wpbu (urgent): w1 | w2 | w3s=(2/D)w3 | wv_r=-(2/D)Wv | w0q=wq@w0
#                 | wkq_t=wq@Wk^T
#  wpbr (rest):   w1^T | w2^T | w3^T | ident
#  stp:           [w0eff=Wk@w0 | S^T]  (split into two DMAs, t0 first)

f32 = mybir.dt.float32
bf16 = mybir.dt.bfloat16

AF = mybir.ActivationFunctionType
ALU = mybir.AluOpType


def _build_program(reps=1):
    nc = bacc.Bacc(
        "TRN2",
        target_bir_lowering=False,
        debug=False,
        enable_asserts=False,
        num_devices=NCORES,
    )

    stp_dr = nc.dram_tensor("stp", [128, D + N], bf16, kind="ExternalInput").ap()
    stm_dr = nc.dram_tensor("s_tmb", [128, N], bf16, kind="ExternalInput").ap()
    wbu_dr = nc.dram_tensor("wpbu", [D, 6 * D], bf16, kind="ExternalInput").ap()
    wbr_dr = nc.dram_tensor("wpbr", [D, 4 * D], bf16, kind="ExternalInput").ap()
    wf_dr = nc.dram_tensor("wpf", [D, 3 * D], f32, kind="ExternalInput").ap()
    out_dr = nc.dram_tensor("out", [128, NT // 128, D], bf16, kind="ExternalOutput").ap()

    with tile.TileContext(nc) as tc:
        for _ in range(reps):
            _emit(tc, stp_dr, stm_dr, wbu_dr, wbr_dr, wf_dr, out_dr)

    nc.compile()
    return nc


def _emit(tc, stp_dr, stm_dr, wbu_dr, wbr_dr, wf_dr, out_dr):
    nc = tc.nc
    from contextlib import ExitStack

    from concourse.tile_rust import add_dep_helper as _dep  # type: ignore

    with ExitStack() as ctx:
        consts = ctx.enter_context(tc.tile_pool(name="consts", bufs=1))
        big = ctx.enter_context(tc.tile_pool(name="big", bufs=1))
        # PSUM: ha0,ha1,hb0,hb1,hc0,hc1 (1 bank each) + stg(2) = 8 banks
        pp = ctx.enter_context(tc.tile_pool(name="pp", bufs=1, space="PSUM"))

        def pbank(tag, name, shape=None, dt=f32):
            return pp.tile(shape or [128, TT], dt, tag=tag, bufs=1, name=name)

        def pstage(name, w=512, dt=f32):
            return pp.tile([128, w], dt, tag="stg", bufs=2, name=name)

        # ---- DMAs, ordered by need (all HWDGE on the sync queue);
        # DMA1 = [w0eff | S^T t0-half] gates the whole forward chain ----
        sta = big.tile([128, D + TT], bf16, tag="sta")   # w0eff | S^T[:, :512]
        stb = big.tile([128, TT], bf16, tag="stb")       # S^T[:, 512:]
        wpbu = consts.tile([D, 6 * D], bf16, tag="wpbu")
        wpbr = consts.tile([D, 4 * D], bf16, tag="wpbr")
        s_tmb = big.tile([128, NCHUNK, 128], bf16, tag="s_tmb")  # token-major
        wpf = consts.tile([D, 3 * D], f32, tag="wpf")
        nc.sync.dma_start(sta[:], stp_dr[:, 0 : D + TT])
        nc.sync.dma_start(stb[:], stp_dr[:, D + TT :])
        nc.sync.dma_start(wpbu[:], wbu_dr)
        nc.sync.dma_start(wpbr[:], wbr_dr)
        nc.sync.dma_start(
            s_tmb[:], stm_dr.rearrange("p (c d) -> p c d", d=128)
        )
        nc.sync.dma_start(wpf[:], wf_dr)

        w0eff = sta[:, 0:D]
        stt = [sta[:, D : D + TT], stb[:]]
        w1b = wpbu[:, ts(0, D)]
        w2b = wpbu[:, ts(1, D)]
        w3s = wpbu[:, ts(2, D)]
        wv_r = wpbu[:, ts(3, D)]
        w0q = wpbu[:, ts(4, D)]                        # wq @ w0
        wkq_t = wpbu[:, ts(5, D)]                      # wq @ Wk^T
        wt = [wpbr[:, ts(i, D)] for i in range(3)]     # w1^T,w2^T,w3^T
        ident_b = wpbr[:, ts(3, D)]
        w_f = [wpf[:, ts(i, D)] for i in range(3)]     # w1,w2,w3 f32

        # tiny scratch silu pulls the first ACT table load off the
        # critical path (runs during the DMAs)
        scr = consts.tile([128, 1], f32, tag="scr")
        scr2 = consts.tile([128, 1], f32, tag="scr2")
        nc.gpsimd.memset(scr[:], 0.0)
        nc.scalar.activation(scr2[:], scr[:], AF.Silu)

        # PE warm-up: start the HAM clock window early so real matmuls
        # hit full clock by ~3us
        wupa = consts.tile([128, 128], f32, tag="wupa")
        nc.gpsimd.memset(wupa[:], 0.0)
        wupp = pstage("wupp")
        for _ in range(4):
            nc.tensor.matmul(
                wupp[:, 0:128], wupa[:], wupa[:],
                skip_group_check=True,
            )

        # per-half persistent SBUF tensors (feature-major, bf16)
        def halves(pfx):
            return [
                big.tile([128, TT], bf16, name=f"{pfx}{t}", tag=f"{pfx}{t}")
                for t in range(NTI)
            ]

        a1 = halves("a1")
        a2 = halves("a2")
        a3 = halves("a3")
        sp0 = halves("sp0")
        sp1 = halves("sp1")
        sp2 = halves("sp2")
        g0 = halves("g0")
        g1 = halves("g1")
        g2 = halves("g2")
        g3 = halves("g3")   # (2/D)(H3 - V)

        silu_insts = []
        dsilu_insts = []

        # ---- forward: six single-bank H tiles; silu -> bf16 halves ----
        hb = {}
        for li, tag in ((0, "ha"), (1, "hb"), (2, "hc")):
            for t in range(NTI):
                hb[li, t] = pbank(f"{tag}{t}", f"h{li}_{t}")
        pt = big.tile([128, NT], bf16, tag="pt")
        l2mm = {}
        for t in range(NTI):
            nc.tensor.matmul(hb[0, t][:], w0eff, stt[t])
            silu_insts.append(nc.scalar.activation(a1[t][:], hb[0, t][:], AF.Silu))
            l2mm[t] = nc.tensor.matmul(hb[1, t][:], w1b, a1[t][:])
            silu_insts.append(nc.scalar.activation(a2[t][:], hb[1, t][:], AF.Silu))
            nc.tensor.matmul(hb[2, t][:], w2b, a2[t][:])
            silu_insts.append(nc.scalar.activation(a3[t][:], hb[2, t][:], AF.Silu))
            # H3 - V accumulated in one stage bank (wv_r is negated+scaled)
            h3 = pstage(f"h3_{t}")
            nc.tensor.matmul(h3[:], wv_r, stt[t], start=True, stop=False)
            nc.tensor.matmul(h3[:], w3s, a3[t][:], start=False, stop=True)
            nc.vector.tensor_copy(g3[t][:], h3[:])
            if t == 0:
                # pt = (wq Wk^T)^T S^T: independent matmul off S^T t0;
                # pinned after L1-t0 so it never sits in the forward's
                # critical PE slot (it's only needed by the backward-era
                # X1 stop matmuls)
                px = pstage("p_pt")
                ptmm = nc.tensor.matmul(px[:], wkq_t, stt[0])
                _dep(ptmm.ins, l2mm[0].ins, sync=False,
                     reason="PE order: forward L1-t0 before pt")
                nc.vector.tensor_copy(pt[:], px[:])

        # ---- dsilu after all silus (one table switch); t0 first so the
        # backward c-mm/mul chain (same order) never queues behind the
        # other half's dsilu ----
        for spd, li in ((sp2, 2), (sp1, 1), (sp0, 0)):
            for t in (0, 1):
                di = nc.scalar.activation(
                    spd[t][:], hb[li, t][:], AF.Derivative_silu
                )
                dsilu_insts.append(di)
        for di in dsilu_insts:
            _dep(di.ins, silu_insts[-1].ins, sync=False, reason="act-table order")

        # ---- backward: tile-1 first (it gates M); c tiles reuse the H
        # banks (freed in dsilu order, which matches mul consumption).
        # Engine split: the t1-half muls go to the idle Pool engine so
        # the M-critical t0 chain never queues on DVE. ----
        # c-bank plan: c2 -> the stage banks (free right after the g3
        # evicts, so the first backward mm never waits a dsilu WAR),
        # c1 -> h2's banks (free after the sp2 dsilus), c0 -> h1's banks.
        cb = {}
        for t in (0, 1):
            cb[2, t] = pstage(f"c2_{t}")
        for li, tag in ((1, "hc"), (0, "hb")):
            for t in (0, 1):
                cb[li, t] = pbank(f"{tag}{t}", f"c{li}_{t}")
        last_cmm = None
        for li, gin, gout, spd in (
            (2, g3, g2, sp2), (1, g2, g1, sp1), (0, g1, g0, sp0)
        ):
            for t in (0, 1):
                last_cmm = nc.tensor.matmul(cb[li, t][:], wt[li], gin[t][:])
                nc.vector.tensor_mul(gout[t][:], cb[li, t][:], spd[t][:])

        # reload the silu table during the dW phase, off the tail
        scr3 = consts.tile([128, 1], f32, tag="scr3")
        dummy = nc.scalar.activation(scr3[:], scr[:], AF.Silu)
        _dep(dummy.ins, dsilu_insts[-1].ins, sync=False, reason="act-table order")

        # ---- token-major copies: XBAR for a1..a3,g3,g2; PE path for g0/g1 --
        a_tm = [None] + [
            big.tile([128, NCHUNK, 128], bf16, name=f"atm{i}", tag=f"atm{i}")
            for i in (1, 2, 3)
        ]
        g_tm = [
            big.tile([128, NCHUNK, 128], bf16, name=f"gtm{i}", tag=f"gtm{i}")
            for i in range(4)
        ]
        for src, dst, horder in (
            (a1, a_tm[1], (0, 1)), (a2, a_tm[2], (0, 1)), (a3, a_tm[3], (0, 1)),
            (g3, g_tm[3], (0, 1)), (g2, g_tm[2], (0, 1)),
        ):
            for h in horder:
                nc.sync.dma_start_transpose(dst[:, 4 * h : 4 * h + 4], src[h][:])

        # g1 feeds dW1 -> u1 -> retrieval layer 2; PE transposes into the
        # freed hb banks, both halves evicted on Pool (order-pinned after
        # the Pool g0t1 mul so M is never delayed).
        pg1 = [None, None]
        for h in (0, 1):
            pg1[h] = pbank(f"hc{h}", f"pg1_{h}", shape=[128, TT], dt=bf16)
            for j in range(4):
                nc.tensor.matmul(
                    pg1[h][:, ts(j, 128)], g1[h][:, ts(j, 128)], ident_b,
                    is_transpose=True,
                )

        # g0: PE transposes + DVE half evicts (lowest latency on the
        # tail); separate stage tiles per half so the t1 transposes don't
        # wait on the t0 eviction (tile-granular WAR). t0's mul lands
        # first, so t0 transposes/evicts/M-chunks all go first.
        pg0 = [pstage(f"p_g0{h}", w=512, dt=bf16) for h in range(NTI)]
        last_pg0 = None
        for h in (0, 1):
            for j in range(4):
                last_pg0 = nc.tensor.matmul(
                    pg0[h][:, ts(j, 128)], g0[h][:, ts(j, 128)], ident_b,
                    is_transpose=True,
                )
        ev_g0h0 = nc.vector.tensor_copy(
            g_tm[0][:, 0:4], pg0[0][:].rearrange("p (c d) -> p c d", d=128)
        )
        ev_g0h1 = nc.vector.tensor_copy(
            g_tm[0][:, 4:8], pg0[1][:].rearrange("p (c d) -> p c d", d=128)
        )
        _dep(ev_g0h1.ins, ev_g0h0.ins, sync=False, reason="DVE order: t0 evict first")

        # pg1 evicts on ACT right after the dsilus and BEFORE the silu
        # reload (Copy is in every table, so no extra load) -- dW1's
        # inputs are then ready by ~14k and u1 lands before px2 needs it;
        # the reload still finishes well before the first retrieval silu
        ev_p1h0 = nc.scalar.activation(
            g_tm[1][:, 0:4].rearrange("p c d -> p (c d)"), pg1[0][:], AF.Copy
        )
        _dep(ev_p1h0.ins, dsilu_insts[-1].ins, sync=False,
             reason="ACT order: dsilus before pg1 evicts")
        ev_p1h1 = nc.scalar.activation(
            g_tm[1][:, 4:8].rearrange("p c d -> p (c d)"), pg1[1][:], AF.Copy
        )
        _dep(ev_p1h1.ins, ev_p1h0.ins, sync=False, reason="ACT order")
        _dep(dummy.ins, ev_p1h1.ins, sync=False,
             reason="ACT order: pg1 evicts before reload")

        # ---- M = S^T G0 in its OWN bank/group so m_r never waits the
        # late dW1 round-trip ----
        macc = pstage("macc", w=128)
        m_stop = None
        for k, c in enumerate((0, 1, 2, 3, 4, 5, 6, 7)):
            m_stop = nc.tensor.matmul(
                macc[:, 0:128],
                s_tmb[:, c],
                g_tm[0][:, c],
                start=(k == 0),
                stop=(c == 7),
            )
            if k == 0:
                _dep(m_stop.ins, last_pg0.ins, sync=False,
                     reason="PE order: M right after g0 transposes")
        m_r = big.tile([128, 128], bf16, tag="m_r")
        nc.vector.tensor_copy(m_r[:], macc[:, 0:128])

        # ---- dW1 in its OWN bank/group: u1 feeds retrieval layer 2 and
        # must not wait behind dW3/dW2's group stop ----
        dw1acc = pstage("dw1acc", w=128)
        dw1_stop = None
        dw1_first = None
        for k, c in enumerate((0, 1, 2, 3, 4, 5, 6, 7)):
            dw1_stop = nc.tensor.matmul(
                dw1acc[:, 0:128],
                a_tm[1][:, c],
                g_tm[1][:, c],
                start=(k == 0),
                stop=(c == 7),
            )
            if k == 0:
                dw1_first = dw1_stop
                _dep(dw1_first.ins, last_cmm.ins, sync=False,
                     reason="PE order: backward before dW")
                _dep(dw1_first.ins, m_stop.ins, sync=False,
                     reason="PE order: M before dW1")

        # ---- dW2 in hb1 (free after c0t1's mul): u2 lands right
        # after X1stop instead of waiting dW3's group ----
        dw2acc = pbank("hb1", "dw2acc", shape=[128, 128])
        dw2_stop = None
        dw2_first = None
        for c in range(NCHUNK):
            dw2_stop = nc.tensor.matmul(
                dw2acc[:],
                a_tm[2][:, c],
                g_tm[2][:, c],
                start=(c == 0),
                stop=(c == NCHUNK - 1),
            )
            if c == 0:
                dw2_first = dw2_stop
                _dep(dw2_first.ins, last_cmm.ins, sync=False,
                     reason="PE order: backward before dW")
                _dep(dw2_first.ins, m_stop.ins, sync=False,
                     reason="PE order: M before dW2")

        # ---- dW3 group (only feeds u3, needed last) ----
        acc = pbank("hc1", "dwacc", shape=[128, 128])
        dw_stop = None
        dw_first = None
        for c in range(NCHUNK):
            dw_stop = nc.tensor.matmul(
                acc[:],
                a_tm[3][:, c],
                g_tm[3][:, c],
                start=(c == 0),
                stop=(c == NCHUNK - 1),
            )
            if dw_first is None:
                dw_first = dw_stop
                _dep(dw_first.ins, last_cmm.ins, sync=False,
                     reason="PE order: backward before dW")
                _dep(dw_first.ins, m_stop.ins, sync=False,
                     reason="PE order: M before dW3")

        u = [None]
        for nm, accb, stop_i, wf in (
            ("u1", dw1acc[:, 0:128], dw1_stop, w_f[0]),
            ("u2", dw2acc[:], dw2_stop, w_f[1]),
            ("u3", acc[:], dw_stop, w_f[2]),
        ):
            ut = big.tile([D, D], bf16, name=nm, tag=nm)
            ai = nc.vector.tensor_add(ut[:], accb, wf)
            # same-bank safety: no reads before the group's stop matmul
            _dep(ai.ins, stop_i.ins, sync=True, reason=f"{nm} bank group")
            u.append(ut)
        # u[1]=w1+dW1, u[2]=w2+dW2, u[3]=w3+dW3

        # ---- retrieval: X1 = (wq w0)^T S^T + M^T P, then layers 2..4 ----
        # symmetric retrieval halves: each half's silu (398) just covers
        # the other half's mm + cross-engine hops (~410), keeping ACT
        # saturated; asymmetric splits open layer-boundary bubbles
        RHS = (RH, RH)
        ROF = (0, RH)
        nh = 2
        r1, r2, r3 = [], [], []
        for h in range(nh):
            r1.append(big.tile([128, RHS[h]], bf16, name=f"r1h{h}", tag=f"r1h{h}"))
            r2.append(big.tile([128, RHS[h]], bf16, name=f"r2h{h}", tag=f"r2h{h}"))
            r3.append(big.tile([128, RHS[h]], bf16, name=f"r3h{h}", tag=f"r3h{h}"))

        px1 = [pbank(f"ha{hh}", f"px1_{hh}", shape=[128, RHS[hh]]) for hh in range(nh)]
        for hh in range(nh):
            # term 1 ((wq w0)^T S^T) has no M dependency -- runs during dW
            t1mm = nc.tensor.matmul(
                px1[hh][:], w0q, stt[0][:, ROF[hh] : ROF[hh] + RHS[hh]],
                start=True, stop=False
            )
            # px1's bank frees after the sp0-t0 dsilu; run term1 in the
            # PE idle gaps of the backward-mul phase
            _dep(t1mm.ins, last_cmm.ins, sync=False,
                 reason="PE order: backward mms before px1 term1")
        for hh in range(nh):
            smm = nc.tensor.matmul(
                px1[hh][:], m_r[:], pt[:, ROF[hh] : ROF[hh] + RHS[hh]],
                start=False, stop=True
            )
            # dW1 fills the PE window while m_r evicts on DVE; the X1
            # stops come right after it, and dW2/dW3 stay behind them
            _dep(smm.ins, dw1_stop.ins, sync=False,
                 reason="PE order: dW1 before X1 stop")
            _dep(dw_first.ins, smm.ins, sync=False,
                 reason="PE order: X1 stop before dW3")
            _dep(dw2_first.ins, smm.ins, sync=False,
                 reason="PE order: X1 stop before dW2")
            nc.scalar.activation(r1[hh][:], px1[hh][:], AF.Silu)
        px2 = [pbank(f"hb{hh}", f"px2_{hh}", shape=[128, RHS[hh]]) for hh in range(nh)]
        for hh in range(nh):
            pmm = nc.tensor.matmul(px2[hh][:], u[1][:], r1[hh][:])
            _dep(dw_first.ins, pmm.ins, sync=False,
                 reason="PE order: px2 before dW3")
            nc.scalar.activation(r2[hh][:], px2[hh][:], AF.Silu)
        px3 = [
            pbank("hc0", "px3_0", shape=[128, RHS[0]]),
            pstage("px3_1", w=RHS[1]),
        ]
        for hh in range(nh):
            nc.tensor.matmul(px3[hh][:], u[2][:], r2[hh][:])
            nc.scalar.activation(r3[hh][:], px3[hh][:], AF.Silu)
        out_r = out_dr  # [p, c, d]: token c*128+p, contiguous per partition
        o_tm = big.tile([128, 4, 128], bf16, tag="o_tm")
        for hh in range(nh):
            po = pstage(f"po{hh}", w=RHS[hh])
            pov = po[:].rearrange("p (c d) -> p c d", d=128)
            nchk = RHS[hh] // 128
            for j in range(nchk):
                nc.tensor.matmul(
                    pov[:, j], r3[hh][:, ts(j, 128)], u[3][:],
                    start=(j == 0), stop=(j == nchk - 1),
                )
            if hh == 0:
                # ACT is free once r3h1's silu retires and h0's po matmuls
                # are long done -- evicting h0 there runs in parallel with
                # DVE's h1 evict
                nc.scalar.activation(
                    o_tm[:, 0:2].rearrange("p c d -> p (c d)"),
                    pov[:].rearrange("p c d -> p (c d)"), AF.Copy
                )
            else:
                nc.vector.tensor_copy(o_tm[:, 2:4], pov[:])
        # single out DMA: one HWDGE gen + one completion-sem wait on the
        # tail (splitting pays a second serial HWDGE gen + sem -- worse)
        nc.sync.dma_start(out_r[:], o_tm[:])


_CACHE = {}


def _get_nc():
    if "nc" not in _CACHE:
        _CACHE["nc"] = _build_program()
    return _CACHE["nc"]


def _bf(x):
    return np.ascontiguousarray(x.astype(ml_dtypes.bfloat16))


def _prep_weights(w0, w1, w2, w3, wq, wkv):
    """Host-side weight-space prep (layout, transposes, scales, composes)."""
    w0, w1, w2, w3, wq, wkv = (
        np.asarray(x, np.float32) for x in (w0, w1, w2, w3, wq, wkv)
    )
    wk, wv = wkv[:, :D], wkv[:, D:]
    ident = np.eye(D, dtype=np.float32)
    w0eff = wk @ w0
    wpbu = np.concatenate(
        [
            w1, w2,
            (2.0 / D) * w3,     # w3s
            (-2.0 / D) * wv,    # wv_r
            wq @ w0,            # w0q: X1 term1 = (wq w0)^T S^T
            wq @ wk.T,          # wkq_t: pt = (wq Wk^T)^T S^T
        ],
        axis=1,
    )
    wpbr = np.concatenate([w1.T, w2.T, w3.T, ident], axis=1)
    wpf = np.ascontiguousarray(np.concatenate([w1, w2, w3], axis=1))
    return _bf(w0eff), _bf(wpbu), _bf(wpbr), wpf


def kernel(seq, w0, w1, w2, w3, wq, wkv):
    nc = _get_nc()
    seq = np.asarray(seq, np.float32)
    w0eff, wpbu, wpbr, wpf = _prep_weights(w0, w1, w2, w3, wq, wkv)

    in_maps = []
    for c in range(NCORES):
        b, h = c // 2, c % 2
        if h == 0:
            s = seq[b]
        else:
            # rotate: retrieval half first; grad sum is order-invariant
            s = np.concatenate([seq[b, NT:], seq[b, :NT]], axis=0)
        sb = s.astype(ml_dtypes.bfloat16)
        # stp = [w0eff | S^T] so the first DMA carries layer-0's weight
        stp = np.ascontiguousarray(
            np.concatenate([w0eff, np.ascontiguousarray(sb.T)], axis=1)
        )
        # token-major [128, c, d] flattened: partition p, token c*128+p
        stm = np.ascontiguousarray(
            sb.reshape(NCHUNK, 128, D).transpose(1, 0, 2).reshape(128, N)
        )
        in_maps.append(
            {
                "stp": stp,
                "s_tmb": stm,
                "wpbu": wpbu,
                "wpbr": wpbr,
                "wpf": wpf,
            }
        )

    res = run_bass_kernel_spmd(nc, in_maps, core_ids=list(range(NCORES)))
    _CACHE["last_results"] = res

    out = np.empty((B, N, D), np.float32)
    for c in range(NCORES):
        b, h = c // 2, c % 2
        # device layout [p, chunk, d] -> tokens (chunk*128+p, d)
        ob = res.results[c]["out"].astype(np.float32)
        out[b, h * NT : (h + 1) * NT] = ob.transpose(1, 0, 2).reshape(NT, D)
    return out
